# revision 27
# baseline (speedup 1.0000x reference)
"""Trainium2 Bass kernel for the time-independent "GRU" problem.

Math (h0 == 0, which setup_inputs guarantees):
    r     = sigmoid(x @ Wr[:I] + br)
    z     = sigmoid(x @ Wz[:I] + bz)
    h1    = tanh(x @ Wh[:I] + (z*r) @ Wh[I:] + bh)
    out   = sigmoid((h1*z) @ Wo + bo)
returned as (out, h0), mirroring the reference.

Strategy: data-parallel over batch (64 -> 8 per core).  Everything is
computed transposed ([H, rows] with H on partitions) so the weight
matrices serve directly as the matmul stationary operand (lhsT = W[K,M])
and biases become per-partition scalars folded into the ScalarE
activation.  x is host-transposed to [I, B*T] per core; matmuls run in
bf16 (2x the fp32 PE rate) with fp32 PSUM accumulation; the final
sigmoid output is stored fp32.
"""

import numpy as np
import ml_dtypes

_B, _T, _I, _H, _O = 64, 512, 512, 512, 512
_NCORES = 8
_BPC = _B // _NCORES          # 8 batch elements per core
_ROWS = _BPC * _T             # 4096 rows (b,t) per core
_FCH = 512                    # moving free-dim chunk (max for fp32 PSUM)
_NCH = _ROWS // _FCH          # 8 chunks
_KT = _I // 128               # 4 contraction tiles
_MT = _H // 128               # 4 output-partition tiles

_BF16 = ml_dtypes.bfloat16

_cache = {}


def _np_reference(x, h0, Wr, br, Wz, bz, Wh, bh, Wo, bo, N):
    """Exact numpy port of the reference (general h0 / N fallback)."""
    I = x.shape[2]
    xt = x[:, :N, :]

    def sig(v):
        return 1.0 / (1.0 + np.exp(-v))

    Wr_x, Wr_h = Wr[:I], Wr[I:]
    Wz_x, Wz_h = Wz[:I], Wz[I:]
    Wh_x, Wh_h = Wh[:I], Wh[I:]
    hr = h0 @ Wr_h
    hz = h0 @ Wz_h
    r = sig(xt @ Wr_x + hr[:, None, :] + br)
    z = sig(xt @ Wz_x + hz[:, None, :] + bz)
    zr = z * r
    h1 = np.tanh(xt @ Wh_x + zr @ Wh_h + bh)
    h_new = (1.0 - z) * h0[:, None, :] + h1 * z
    out = sig(h_new @ Wo + bo)
    return out.astype(np.float32), np.asarray(h0, dtype=np.float32)


def _build():
    import concourse.bass as bass
    import concourse.mybir as mybir
    from concourse import bacc
    from concourse.tile import TileContext

    bf16 = mybir.dt.bfloat16
    f32 = mybir.dt.float32
    ts, ds = bass.ts, bass.ds
    ACT = mybir.ActivationFunctionType

    nc = bacc.Bacc()
    xT_ext = nc.declare_dram_parameter("xT", [_I, _ROWS], bf16, isOutput=False)
    w_ext = {
        name: nc.declare_dram_parameter(name, [_I, _H], bf16, isOutput=False)
        for name in ("wrx", "wzx", "whx", "whh", "wo")
    }
    bias_ext = nc.declare_dram_parameter("bias", [128, 16], f32, isOutput=False)
    outT_ext = nc.declare_dram_parameter("outT", [_O, _ROWS], f32, isOutput=True)

    with TileContext(nc) as tc:
        with (
            tc.tile_pool(name="consts", bufs=1) as consts,
            tc.tile_pool(name="xin", bufs=3) as xin,
            tc.tile_pool(name="acts", bufs=2) as acts,
            tc.tile_pool(name="outp", bufs=3) as outp,
            tc.tile_pool(name="ps", bufs=8, space="PSUM") as ps,
        ):
            # Preload the ACT spline tables (sigmoid/tanh) while the weight
            # DMAs are in flight so the first real activation doesn't pay
            # the table-load latency on the critical path.
            warm = consts.tile([128, 1], f32)
            nc.vector.memset(warm, 0.0)
            warm2 = consts.tile([128, 1], f32)
            nc.scalar.activation(warm2, warm, ACT.Sigmoid)
            nc.scalar.activation(warm2, warm, ACT.Tanh)

            # Spend the PE's cold-clock (HAM) ramp on dummy matmuls during
            # the initial DMA window so the real matmuls start warm.
            wmm_w = consts.tile([128, 128], bf16)
            nc.vector.memset(wmm_w, 0.0)
            for _ in range(24):
                wps = ps.tile([128, _FCH], f32, tag="ps")
                nc.tensor.matmul(wps[:, :128], wmm_w, wmm_w, start=True, stop=True)

            # DMA issue order = critical-path order: the first matmuls need
            # only wrx's k0 slice + the first x chunk; bias gates the first
            # sigmoid; the remaining weights follow in use order.
            w_sb = {}
            wrx_t = consts.tile([128, _KT, _H], bf16, tag="wrx", name="wrx_t")
            w_sb["wrx"] = wrx_t

            # half-width first chunk (faster pipeline fill) and last chunk
            # (faster drain, with per-m stores instead of one merged store)
            chunks = [(0, 256), (256, 256)]
            chunks += [(c, 512) for c in range(512, _ROWS, 512)]

            xt_tiles = {}
            xt0 = xin.tile([128, _KT, _FCH], bf16, tag="xt", name="xt0")
            xt_tiles[0] = xt0
            f00, fw0 = chunks[0]
            nc.sync.dma_start(
                out=xt_tiles[0][:, :, :fw0],
                in_=xT_ext[:, ds(f00, fw0)].rearrange("(ko p) n -> p ko n", p=128),
            )
            nc.sync.dma_start(out=wrx_t[:, 0], in_=w_ext["wrx"][ts(0, 128), :])
            bias_sb = consts.tile([128, 16], f32)
            nc.sync.dma_start(out=bias_sb, in_=bias_ext[:, :])
            nc.sync.dma_start(
                out=wrx_t[:, 1:],
                in_=w_ext["wrx"][ds(128, 384), :].rearrange(
                    "(ko p) m -> p ko m", p=128
                ),
            )
            for name in ("wzx", "whx", "whh", "wo"):
                wt = consts.tile([128, _KT, _H], bf16, tag=name)
                nc.sync.dma_start(
                    out=wt, in_=w_ext[name].rearrange("(ko p) m -> p ko m", p=128)
                )
                w_sb[name] = wt

            def _emit_out(fs, fw, hn_t, per_m_store):
                # out = sigmoid(Wo.T @ hn + bo); merged store per chunk
                # except at the drain tail (per-m stores overlap the final
                # activation chain)
                ot = outp.tile([128, _MT, _FCH], f32, tag="ot", name="ot")
                for m in range(_MT):
                    po = ps.tile([128, _FCH], f32, tag="ps", name="po")
                    for k in range(_KT):
                        nc.tensor.matmul(
                            po[:, :fw],
                            w_sb["wo"][:, k, ts(m, 128)],
                            hn_t[k][:, :fw],
                            start=(k == 0),
                            stop=(k == _KT - 1),
                        )
                    nc.scalar.activation(
                        ot[:, m, :fw],
                        po[:, :fw],
                        ACT.Sigmoid,
                        bias=bias_sb[:, ds(12 + m, 1)],
                    )
                    if per_m_store:
                        nc.sync.dma_start(
                            out=outT_ext[ts(m, 128), fs], in_=ot[:, m, :fw]
                        )
                if not per_m_store:
                    nc.sync.dma_start(
                        out=outT_ext[:, fs].rearrange("(m p) n -> p m n", p=128),
                        in_=ot[:, :, :fw],
                    )

            pending_o = None
            for ci, (f0, fw) in enumerate(chunks):
                fs = ds(f0, fw)
                if ci in xt_tiles:
                    xt = xt_tiles[ci]
                else:
                    xt = xin.tile([128, _KT, _FCH], bf16, tag="xt")
                    nc.sync.dma_start(
                        out=xt[:, :, :fw],
                        in_=xT_ext[:, fs].rearrange("(ko p) n -> p ko n", p=128),
                    )

                r_t, z_t, zr_t, hn_t = [], [], [], []
                # r and z gates: sigmoid(W.T @ x + b).  Matmuls are emitted
                # k-outer so the k0 work can start as soon as the first
                # weight slice has landed (kernel start).
                for gate, wname, bcol, lst in (
                    ("r", "wrx", 0, r_t),
                    ("z", "wzx", 4, z_t),
                ):
                    pgs = []
                    for m in range(_MT):
                        pg = ps.tile([128, _FCH], f32, tag="ps")
                        pgs.append(pg)
                    for k in range(_KT):
                        for m in range(_MT):
                            nc.tensor.matmul(
                                pgs[m][:, :fw],
                                w_sb[wname][:, k, ts(m, 128)],
                                xt[:, k, :fw],
                                start=(k == 0),
                                stop=(k == _KT - 1),
                            )
                    for m in range(_MT):
                        gt = acts.tile([128, _FCH], bf16, tag=f"{gate}{m}")
                        nc.scalar.activation(
                            gt[:, :fw],
                            pgs[m][:, :fw],
                            ACT.Sigmoid,
                            bias=bias_sb[:, ds(bcol + m, 1)],
                        )
                        lst.append(gt)
                for m in range(_MT):
                    zr = acts.tile([128, _FCH], bf16, tag=f"zr{m}")
                    nc.vector.tensor_mul(zr[:, :fw], z_t[m][:, :fw], r_t[m][:, :fw])
                    zr_t.append(zr)
                # deferred out-phase of the previous chunk: fills the PE
                # while this chunk's zr sigmoid/mul chain completes
                if pending_o is not None:
                    _emit_out(*pending_o)
                    pending_o = None
                # h1 = tanh(Whx.T @ x + Whh.T @ zr + bh); hn = h1*z
                for m in range(_MT):
                    ph = ps.tile([128, _FCH], f32, tag="ps")
                    for k in range(_KT):
                        nc.tensor.matmul(
                            ph[:, :fw],
                            w_sb["whx"][:, k, ts(m, 128)],
                            xt[:, k, :fw],
                            start=(k == 0),
                            stop=False,
                        )
                    for k in range(_KT):
                        nc.tensor.matmul(
                            ph[:, :fw],
                            w_sb["whh"][:, k, ts(m, 128)],
                            zr_t[k][:, :fw],
                            start=False,
                            stop=(k == _KT - 1),
                        )
                    h1 = acts.tile([128, _FCH], bf16, tag=f"h1{m}")
                    nc.scalar.activation(
                        h1[:, :fw], ph[:, :fw], ACT.Tanh, bias=bias_sb[:, ds(8 + m, 1)]
                    )
                    hn = acts.tile([128, _FCH], bf16, tag=f"hn{m}")
                    nc.vector.tensor_mul(hn[:, :fw], h1[:, :fw], z_t[m][:, :fw])
                    hn_t.append(hn)
                # out-phase is deferred into the next chunk's zr window
                if ci == len(chunks) - 1:
                    _emit_out(fs, fw, hn_t, True)
                else:
                    pending_o = (fs, fw, hn_t, False)

    nc.finalize()
    return nc


def _get_nc():
    if "nc" not in _cache:
        _cache["nc"] = _build()
    return _cache["nc"]


def kernel(x, h0, Wr, br, Wz, bz, Wh, bh, Wo, bo, N):
    x = np.asarray(x)
    h0 = np.asarray(h0, dtype=np.float32)
    N = int(N)
    if (
        x.shape != (_B, _T, _I)
        or N != _T
        or h0.shape != (_B, _H)
        or np.any(h0)
    ):
        return _np_reference(x, h0, Wr, br, Wz, bz, Wh, bh, Wo, bo, N)

    from concourse.bass_utils import run_bass_kernel_spmd

    nc = _get_nc()

    wrx = np.ascontiguousarray(np.asarray(Wr)[:_I]).astype(_BF16)
    wzx = np.ascontiguousarray(np.asarray(Wz)[:_I]).astype(_BF16)
    whx = np.ascontiguousarray(np.asarray(Wh)[:_I]).astype(_BF16)
    whh = np.ascontiguousarray(np.asarray(Wh)[_I:]).astype(_BF16)
    wo = np.ascontiguousarray(np.asarray(Wo)).astype(_BF16)
    bias = np.stack(
        [np.asarray(b, dtype=np.float32).reshape(4, 128) for b in (br, bz, bh, bo)],
        axis=0,
    )  # [4 gates, 4 chunks, 128]
    bias = bias.reshape(16, 128).T.copy()  # [128, 16]; col = gate*4 + chunk

    in_maps = []
    for c in range(_NCORES):
        xc = x[c * _BPC : (c + 1) * _BPC]  # [8, 512, 512]
        xT = xc.transpose(2, 0, 1).reshape(_I, _ROWS).astype(_BF16)
        in_maps.append(
            {
                "xT": xT,
                "wrx": wrx,
                "wzx": wzx,
                "whx": whx,
                "whh": whh,
                "wo": wo,
                "bias": bias,
            }
        )

    res = run_bass_kernel_spmd(nc, in_maps, list(range(_NCORES)))

    out = np.empty((_B, _T, _O), dtype=np.float32)
    for c in range(_NCORES):
        outT = res.results[c]["outT"]  # [512, 4096]
        out[c * _BPC : (c + 1) * _BPC] = outT.reshape(_O, _BPC, _T).transpose(
            1, 2, 0
        )
    return out, h0
